# revision 1
# baseline (speedup 1.0000x reference)
"""Trainium2 Bass kernel for nn_Compressor (sparse_attention block compressor).

Math (reference):
  proj = x @ [W_kv; W_gate]^T            # [b*s, 2048]
  kv   = proj[:, :1024] + ape[s%4]       # blockwise (RATIO=4) abs-pos bias
  sc   = proj[:, 1024:]
  window(blk) = {prev blk rows, ch 0:512} + {cur blk rows, ch 512:1024}
  pooled[blk, c] = softmax-gated channelwise pool over the 8-entry window
  out = (RMSNorm(pooled) -> rope on ch 448:512) @ H  (512x512 Hadamard)

Distribution: 8 cores, data-parallel over (batch, seq-half). Each core owns
2048 seq rows = 512 blocks; the 1-block halo is handled by shifting the
matmul rhs window by 4 rows (xs input carries 16 halo rows).

Key implementation tricks:
  * x^T in bf16 obtained directly by DMA-transposing x viewed as uint16 and
    keeping only the hi-16 planes (= truncate-to-bf16 for free).
  * Projections: W^T tiles stationary (lhsT), x^T moving -> PSUM layout
    [channels(part), m(free)], so the whole softmax pooling is free-axis
    DVE/ACT work and the halo is a free-axis slice offset.
  * Softmax without max-subtraction (scores are ~N(0,1.3); fp32 exp cannot
    overflow; block-0 masking is a 0/1 multiply on exp with a per-core mask).
  * RMSNorm channel reduction via a ones-vector matmul; the scale is applied
    per-partition after the final Hadamard matmul (everything in between is
    linear).
  * Rope pair swap via a tiny permutation matmul; cos/sin tables precomputed
    on host per core.
"""

import os
import numpy as np
import ml_dtypes

import concourse.bass as bass
import concourse.bacc as bacc
import concourse.mybir as mybir
from concourse.tile import TileContext
from concourse.bass_utils import run_bass_kernel_spmd

BF16 = ml_dtypes.bfloat16
F32 = mybir.dt.float32
BF = mybir.dt.bfloat16

N_CORES = 8
DIM = 4096
OCH = 2048          # kv 1024 + gate 1024
ROWS = 2048         # own rows per core
XS_ROWS = 2064      # 16 halo/pad rows + 2048
MCH = 4             # m-chunks per core
MROWS = 512         # rows per m-chunk
NBLK = 128          # blocks per m-chunk
DCH = 32            # d chunks of 128
OCHK = 16           # o chunks of 128
# o-chunks 0..3 kv-first(prev), 4..7 kv-second(cur), 8..11 sc-first, 12..15 sc-second
FIRST_HALF = (0, 1, 2, 3, 8, 9, 10, 11)

_CACHE = {}


def _build():
    nc = bacc.Bacc("TRN2", target_bir_lowering=False, debug=False,
                   num_devices=N_CORES)
    xs = nc.dram_tensor("xs", [DIM, XS_ROWS], BF, kind="ExternalInput")
    wp = nc.dram_tensor("wp", [OCHK, 128, DCH, 128], BF, kind="ExternalInput")
    ape_d = nc.dram_tensor("ape_t", [128, 32], F32, kind="ExternalInput")
    cos_d = nc.dram_tensor("cos_t", [128, 512], F32, kind="ExternalInput")
    sin_d = nc.dram_tensor("sin_t", [128, 512], F32, kind="ExternalInput")
    psw_d = nc.dram_tensor("psw", [128, 128], F32, kind="ExternalInput")
    h_d = nc.dram_tensor("hmat", [128, 4, 512], F32, kind="ExternalInput")
    zmask_d = nc.dram_tensor("zmask", [128, 1], F32, kind="ExternalInput")
    out_d = nc.dram_tensor("out", [4 * NBLK, 512], F32, kind="ExternalOutput")

    with TileContext(nc) as tc:
        with (
            tc.tile_pool(name="const", bufs=1) as constp,
            tc.tile_pool(name="xt", bufs=2) as xtp,
            tc.tile_pool(name="wt", bufs=3) as wtp,
            tc.tile_pool(name="sb", bufs=2) as sbp,
            tc.tile_pool(name="pl", bufs=2) as plp,
            tc.tile_pool(name="sm", bufs=2) as smp,
            tc.tile_pool(name="osb", bufs=2) as outp,
            tc.tile_pool(name="proj", bufs=4, space="PSUM") as projp,
            tc.tile_pool(name="had", bufs=2, space="PSUM") as hadp,
            tc.tile_pool(name="aux", bufs=1, space="PSUM") as auxp,
        ):
            # ---- constants ----
            ape_sb = constp.tile([128, 32], F32, tag="ape")
            nc.gpsimd.dma_start(out=ape_sb[:], in_=ape_d[:, :])
            cos_sb = constp.tile([128, 512], F32, tag="cos")
            nc.gpsimd.dma_start(out=cos_sb[:], in_=cos_d[:, :])
            sin_sb = constp.tile([128, 512], F32, tag="sin")
            nc.gpsimd.dma_start(out=sin_sb[:], in_=sin_d[:, :])
            psw_sb = constp.tile([128, 128], F32, tag="psw")
            nc.gpsimd.dma_start(out=psw_sb[:], in_=psw_d[:, :])
            h_sb = constp.tile([128, 4, 512], F32, tag="h")
            nc.gpsimd.dma_start(out=h_sb[:], in_=h_d[:, :, :])
            zmask_sb = constp.tile([128, 1], F32, tag="zmask")
            nc.gpsimd.dma_start(out=zmask_sb[:], in_=zmask_d[:, :])
            ones_sb = constp.tile([128, 1], F32, tag="ones")
            nc.vector.memset(ones_sb[:], 1.0)
            eps_sb = constp.tile([128, 1], F32, tag="eps")
            nc.vector.memset(eps_sb[:], 1e-6)

            for mch in range(MCH):
                r0 = MROWS * mch
                # ---- x^T tile: [128(d), 32 dchunk, 528 m] bf16; xs is the
                # host-pre-transposed trunc-bf16 x^T (slot s <-> own row
                # r0 + s - 16; slots 12..15 = halo rows r0-4..r0-1).
                xt = xtp.tile([128, DCH, 528], BF, tag="xt")
                for c in range(DCH):
                    nc.sync.dma_start(
                        out=xt[:, c, :],
                        in_=xs[128 * c:128 * (c + 1), r0:r0 + 528],
                    )

                group = {}  # role -> sbuf tile for the current group j
                pooled = plp.tile([128, 4, NBLK], F32, tag="pooled")
                for j in range(4):
                    for t, oc in enumerate((j, j + 4, j + 8, j + 12)):
                        w = wtp.tile([128, DCH, 128], BF, tag="w")
                        nc.gpsimd.dma_start(out=w[:], in_=wp[oc])
                        ps = projp.tile([128, MROWS], F32, tag="proj")
                        off = 12 if oc in FIRST_HALF else 16
                        for d in range(DCH):
                            nc.tensor.matmul(
                                ps[:],
                                lhsT=w[:, d, :],
                                rhs=xt[:, d, off:off + MROWS],
                                start=(d == 0),
                                stop=(d == DCH - 1),
                            )
                        if oc < 8:
                            # kv chunk: PSUM -> SBUF with ape bias added
                            kv = sbp.tile([128, MROWS], F32, tag=f"kv{t}")
                            a = oc  # ape chunk = kv o-chunk (0..7)
                            ape_ap = (ape_sb[:, 4 * a:4 * a + 4]
                                      .unsqueeze(1).to_broadcast((128, NBLK, 4)))
                            nc.vector.tensor_add(
                                kv[:].rearrange("p (b r) -> p b r", r=4),
                                ps[:].rearrange("p (b r) -> p b r", r=4),
                                ape_ap,
                            )
                            group[f"kv{t}"] = kv
                        else:
                            # score chunk: e = exp(psum) straight to SBUF
                            e = sbp.tile([128, MROWS], F32, tag=f"e{t}")
                            nc.scalar.activation(
                                e[:], ps[:], mybir.ActivationFunctionType.Exp)
                            if mch == 0 and oc < 12:
                                # block-0 of even cores: zero the 4 prev-window
                                # weights (zmask = 0 even / 1 odd)
                                nc.vector.tensor_scalar_mul(
                                    e[:, 0:4], e[:, 0:4], zmask_sb[:, 0:1])
                            group[f"e{t}"] = e

                    kv1, kv2 = group["kv0"], group["kv1"]
                    e1, e2 = group["e2"], group["e3"]
                    X = mybir.AxisListType.X

                    def g4(tile_ap):
                        return tile_ap.rearrange("p (b r) -> p b r", r=4)

                    s1 = smp.tile([128, NBLK], F32, tag="s1")
                    nc.vector.reduce_sum(s1[:], g4(e1[:]), axis=X)
                    s2 = smp.tile([128, NBLK], F32, tag="s2")
                    nc.vector.reduce_sum(s2[:], g4(e2[:]), axis=X)
                    ssum = smp.tile([128, NBLK], F32, tag="ssum")
                    nc.vector.tensor_add(ssum[:], s1[:], s2[:])

                    pm = sbp.tile([128, MROWS], F32, tag="pm")
                    nc.vector.tensor_mul(pm[:], e1[:], kv1[:])
                    q1 = smp.tile([128, NBLK], F32, tag="q1")
                    nc.vector.reduce_sum(q1[:], g4(pm[:]), axis=X)
                    pm2 = sbp.tile([128, MROWS], F32, tag="pm2")
                    nc.vector.tensor_mul(pm2[:], e2[:], kv2[:])
                    q2 = smp.tile([128, NBLK], F32, tag="q2")
                    nc.vector.reduce_sum(q2[:], g4(pm2[:]), axis=X)
                    qsum = smp.tile([128, NBLK], F32, tag="qsum")
                    nc.vector.tensor_add(qsum[:], q1[:], q2[:])

                    rinv = smp.tile([128, NBLK], F32, tag="rinv")
                    nc.vector.reciprocal(rinv[:], ssum[:])
                    nc.vector.tensor_mul(pooled[:, j, :], qsum[:], rinv[:])

                # ---- RMSNorm stats: var_sum[b] = sum_c pooled^2 ----
                sq = plp.tile([128, 4 * NBLK], F32, tag="sq")
                nc.scalar.activation(
                    sq[:], pooled[:].rearrange("p c b -> p (c b)"),
                    mybir.ActivationFunctionType.Square)
                ns_ps = auxp.tile([1, 4 * NBLK], F32, tag="ns")
                nc.tensor.matmul(ns_ps[:], lhsT=ones_sb[:, 0:1], rhs=sq[:],
                                 start=True, stop=True)
                var_row = smp.tile([1, NBLK], F32, tag="var_row")
                nc.vector.reduce_sum(
                    var_row[:],
                    ns_ps[0:1, :].rearrange("p (c b) -> p b c", c=4),
                    axis=mybir.AxisListType.X)
                sd_row = smp.tile([1, NBLK], F32, tag="sd_row")
                nc.scalar.activation(sd_row[:], var_row[:],
                                     mybir.ActivationFunctionType.Sqrt,
                                     scale=1.0 / 512.0, bias=eps_sb[0:1, 0:1])
                scale_row = smp.tile([1, NBLK], F32, tag="scale_row")
                nc.vector.reciprocal(scale_row[:], sd_row[:])
                scale_col = smp.tile([128, 1], F32, tag="scale_col")
                nc.gpsimd.dma_start(out=scale_col[:, 0:1], in_=scale_row[0:1, :])

                # ---- rope on chunk 3 (channels 384..511; rows 64.. are rope) ----
                sw_ps = auxp.tile([128, NBLK], F32, tag="swap")
                nc.tensor.matmul(sw_ps[:], lhsT=psw_sb[:], rhs=pooled[:, 3, :],
                                 start=True, stop=True)
                cslice = cos_sb[:, mch * NBLK:(mch + 1) * NBLK]
                sslice = sin_sb[:, mch * NBLK:(mch + 1) * NBLK]
                tmpc = smp.tile([128, NBLK], F32, tag="tmpc")
                nc.vector.tensor_mul(tmpc[:], pooled[:, 3, :], cslice)
                tmps = smp.tile([128, NBLK], F32, tag="tmps")
                nc.vector.tensor_mul(tmps[:], sw_ps[:], sslice)
                nc.vector.tensor_add(pooled[:, 3, :], tmpc[:], tmps[:])

                # ---- Hadamard: out[b, c'] = sum_c pooled[c, b] H[c, c'] ----
                had_ps = hadp.tile([128, 512], F32, tag="had")
                for j in range(4):
                    nc.tensor.matmul(had_ps[:], lhsT=pooled[:, j, :],
                                     rhs=h_sb[:, j, :],
                                     start=(j == 0), stop=(j == 3))
                out_sb = outp.tile([128, 512], F32, tag="out")
                nc.scalar.activation(out_sb[:], had_ps[:],
                                     mybir.ActivationFunctionType.Copy,
                                     scale=scale_col[:, 0:1])
                nc.gpsimd.dma_start(
                    out=out_d[mch * NBLK:(mch + 1) * NBLK, :], in_=out_sb[:])
    nc.compile()
    return nc


def _prep_shared(W_kv, W_gate, ape, norm_w, H):
    W = np.concatenate([W_kv, W_gate], axis=0).astype(np.float32)  # [2048, 4096]
    Wb = W.astype(BF16)
    wp = np.ascontiguousarray(
        Wb.T.reshape(DCH, 128, OCHK, 128).transpose(2, 1, 0, 3))  # [16,128,32,128]
    ape_t = np.ascontiguousarray(
        ape.astype(np.float32).T.reshape(8, 128, 4).transpose(1, 0, 2)
    ).reshape(128, 32)
    psw = np.zeros((128, 128), np.float32)
    idx = np.arange(64)
    psw[idx, idx] = 1.0
    k2 = np.arange(0, 64, 2)
    psw[64 + k2 + 1, 64 + k2] = 1.0
    psw[64 + k2, 64 + k2 + 1] = 1.0
    hm = np.ascontiguousarray(
        (norm_w.astype(np.float32)[:, None] * H.astype(np.float32))
        .reshape(4, 128, 512).transpose(1, 0, 2))
    return wp, ape_t, psw, hm


def _hadamard(n):
    h = np.array([[1.0]], dtype=np.float32)
    while h.shape[0] < n:
        h = np.block([[h, h], [h, -h]])
    return (h / np.sqrt(n)).astype(np.float32)


def _make_in_maps(x, W_kv, W_gate, ape, norm_w, freqs_cis):
    b, s, _ = x.shape
    H = _hadamard(512)
    wp, ape_t, psw, hm = _prep_shared(W_kv, W_gate, ape, norm_w, H)

    # truncate-to-bf16 (hi-16 planes of the f32 words) and transpose once
    xh = x.reshape(b * s, DIM).view(BF16)[:, 1::2]
    xT = np.ascontiguousarray(xh.T)  # [4096, 16384]
    fr = freqs_cis[:, :, 0]  # [nb, 32]
    fi = freqs_cis[:, :, 1]

    in_maps = []
    for c in range(N_CORES):
        batch, half = c // 2, c % 2
        R0 = batch * s + half * ROWS
        xs = np.zeros((DIM, XS_ROWS), BF16)
        xs[:, 16:] = xT[:, R0:R0 + ROWS]
        if half == 1:
            xs[:, :16] = xT[:, R0 - 16:R0]

        g0 = half * 512
        bi = np.arange(g0, g0 + 512)
        cos_t = np.zeros((128, 512), np.float32)
        cos_t[:64] = 1.0
        cos_t[64:] = np.repeat(fr[bi].T, 2, axis=0)
        sin_t = np.zeros((128, 512), np.float32)
        st = np.repeat(fi[bi].T, 2, axis=0)
        st[0::2] *= -1.0
        sin_t[64:] = st

        zmask = np.full((128, 1), 0.0 if half == 0 else 1.0, np.float32)
        in_maps.append({
            "xs": xs, "wp": wp, "ape_t": ape_t,
            "cos_t": cos_t, "sin_t": sin_t, "psw": psw,
            "hmat": hm, "zmask": zmask,
        })
    return in_maps


def kernel(x, W_kv, W_gate, ape, norm_w, freqs_cis, start_pos=0):
    x = np.asarray(x, dtype=np.float32)
    W_kv = np.asarray(W_kv, dtype=np.float32)
    W_gate = np.asarray(W_gate, dtype=np.float32)
    ape = np.asarray(ape, dtype=np.float32)
    norm_w = np.asarray(norm_w, dtype=np.float32)
    freqs_cis = np.asarray(freqs_cis, dtype=np.float32)

    b, s, _ = x.shape
    nb = s // 4
    assert (b, s) == (4, 4096), (b, s)

    if "nc" not in _CACHE:
        _CACHE["nc"] = _build()
    nc = _CACHE["nc"]

    in_maps = _make_in_maps(x, W_kv, W_gate, ape, norm_w, freqs_cis)

    trace = os.environ.get("KERNEL_TRACE", "") not in ("", "0")
    res = run_bass_kernel_spmd(nc, in_maps, core_ids=list(range(N_CORES)),
                               trace=trace)
    kernel.last_results = res
    out = np.concatenate([res.results[c]["out"] for c in range(N_CORES)], axis=0)
    return np.ascontiguousarray(out.reshape(b, nb, 512))

